# revision 19
# baseline (speedup 1.0000x reference)
"""KAN basis-linear kernel for 8 TRN2 NeuronCores — fp8 DoubleRow edition.

Computes, for x:[B,I], spline_weight:[O,I,K=9], base_weight:[O,I], bias:[O]:

    basis = relu(1 - |(clip(x,-2,2)[...,None] - grid) / delta|)   # hat basis
    out   = einsum('bik,oik->bo', basis, spline_weight)
          + silu(x) @ base_weight.T + bias

Data-parallel over the batch across 8 cores (weights replicated).

Algebra: with grid g_k = -2 + 0.5k, Abel summation turns the hat-basis
contraction into 8 saturating-ramp channels psi_j = clip(2(g_{j+1}-x),0,1);
shifting to phi'_j = psi_j - 0.5 (saturated values +-0.5, exactly
representable in fp8) folds a constant into the bias. The 8 ramp channels
plus a clip(x)/4 error-absorber channel run as fp8(e4m3) DoubleRow matmuls
(2 contraction rows/cycle = 2x bf16 rate), pairing the 18 channels of each
two-i-chunk phi tile into 9 pairs; the silu base branch stays bf16.

fp8 weight quantization is chosen per (o,i) by exact 256-path search: the
output error equals sum_i [P_i(x) - absorber_i(x)] where P_i linearly
interpolates the cumulative quantization errors over the 8 knots; each
(o,i) row has 2^8 floor/ceil choices, scored by a Gram-metric quadratic
form under the empirical x distribution. The best absorber component over
{1, clip(x), silu(x)} is folded into the bias, the clip(x) channel weights
(lambda), and the silu channel weights (mu) — the latter two for free.
"""
import numpy as np
import ml_dtypes
from contextlib import ExitStack

import concourse.bass as bass
import concourse.tile as tile
import concourse.mybir as mybir
from concourse import bacc
from concourse.bass_utils import run_bass_kernel_spmd

N_CORES = 8
B, I, O = 16384, 1024, 1024
B_CORE = B // N_CORES            # 2048 batch rows per core
B_SUPER = 512                    # batch stripe held in PSUM (1 bank per o-tile)
N_SUPERS = B_CORE // B_SUPER     # 4
P = 128
N_ICHK = 8                       # contraction chunks over i
N_T2 = 4                         # phi tiles per stripe (2 i-chunks each)
N_PAIR = 9                       # DoubleRow pairs per 2-ichk tile (18 channels)
N_OT = O // P                    # 8 output tiles (one PSUM bank each)
ALPHA = 32.0                     # fp8 weight scale (psum divided back on evac)
XCS = 4.0                        # clip(x) channel prescale: phi_xc = xc/XCS

F32 = mybir.dt.float32
BF16 = mybir.dt.bfloat16
F8 = mybir.dt.float8e4
E4M3 = ml_dtypes.float8_e4m3
NP_BF16 = ml_dtypes.bfloat16
AF = mybir.ActivationFunctionType
ALU = mybir.AluOpType
DR = mybir.MatmulPerfMode.DoubleRow

_CACHE = {}


def _build():
    nc = bacc.Bacc("TRN2", target_bir_lowering=False, debug=False,
                   num_devices=N_CORES)
    # x tiled on host: [bs, ichk, p, b]
    xt = nc.dram_tensor("xt", [N_SUPERS, N_ICHK, P, B_SUPER], F32,
                        kind="ExternalInput").ap()
    # fp8 weights: [p, 36 pairs, 2, o]; pair t2*9+tp slot r is channel
    # cc=2*tp+r of tile t2: (ichk=2*t2+cc//9, ch=cc%9), ch 0-7 ramps, 8 xc
    wr = nc.dram_tensor("wr", [P, N_T2 * N_PAIR, 2, O], F8,
                        kind="ExternalInput").ap()
    # bf16 silu weights: [p, ichk, o]
    wb = nc.dram_tensor("wb", [P, N_ICHK, O], BF16,
                        kind="ExternalInput").ap()
    bias = nc.dram_tensor("bias", [O], F32, kind="ExternalInput").ap()
    # output tiled: [bs, p, ot, b] (one contiguous 2MB store per stripe)
    outT = nc.dram_tensor("outT", [N_SUPERS, P, N_OT, B_SUPER], F32,
                          kind="ExternalOutput").ap()

    with tile.TileContext(nc) as tc, ExitStack() as ctx:
        const_pool = ctx.enter_context(tc.tile_pool(name="const", bufs=1))
        x_pool = ctx.enter_context(tc.tile_pool(name="xin", bufs=3))
        t_pool = ctx.enter_context(tc.tile_pool(name="tmp", bufs=4))
        phi8_pool = ctx.enter_context(tc.tile_pool(name="phi8", bufs=3))
        phib_pool = ctx.enter_context(tc.tile_pool(name="phib", bufs=4))
        w_pool = ctx.enter_context(tc.tile_pool(name="wts", bufs=1))
        out_pool = ctx.enter_context(tc.tile_pool(name="outs", bufs=2))
        psum_pool = ctx.enter_context(
            tc.tile_pool(name="psum", bufs=N_OT, space="PSUM"))

        # first x chunk ASAP (phi production is the kernel's lead-in)
        x_first = x_pool.tile([P, B_SUPER], F32, tag="xin")
        nc.scalar.dma_start(x_first[:], xt[0, 0])

        # ACT bias constants: 2*g_{j+1} = j - 3 for j=0..7
        consts = const_pool.tile([P, 8], F32)
        for j in range(8):
            nc.any.memset(consts[:, j:j + 1], float(j - 3))

        # bias[o] -> [128, 8] with o = ot*128 + p
        bias_sb = const_pool.tile([P, N_OT], F32)
        nc.scalar.dma_start(bias_sb[:], bias.rearrange("(ot p) -> p ot", p=P))

        # resident weights, streamed in consumption order on the sync queue
        wr_sb = w_pool.tile([P, N_T2 * N_PAIR, 2, O], F8)
        wb_sb = w_pool.tile([P, N_ICHK, O], BF16)
        for tp in range(N_PAIR):  # first tile per-pair so matmul 0 starts early
            nc.sync.dma_start(wr_sb[:, tp:tp + 1], wr[:, tp:tp + 1])
        nc.sync.dma_start(wb_sb[:, 0:2], wb[:, 0:2])
        for t2 in range(1, N_T2):
            nc.sync.dma_start(wr_sb[:, t2 * N_PAIR:(t2 + 1) * N_PAIR],
                              wr[:, t2 * N_PAIR:(t2 + 1) * N_PAIR])
            nc.sync.dma_start(wb_sb[:, 2 * t2:2 * t2 + 2], wb[:, 2 * t2:2 * t2 + 2])

        # Small PE warm-up spin bridging the first input-DMA wait: starts
        # the HAM busy-streak early so the clock-gate reaches 8/8 sooner.
        warm = const_pool.tile([P, B_SUPER], BF16)
        nc.any.memset(warm[:], 0.0)
        warm_ps = psum_pool.tile([P, B_SUPER], F32, tag="psum")
        for _ in range(10):
            nc.tensor.matmul(warm_ps[:], lhsT=warm[:, :P], rhs=warm[:],
                             start=True, stop=True)

        for bs in range(N_SUPERS):
            psums = [psum_pool.tile([P, B_SUPER], F32, tag="psum",
                                    name=f"psum_{bs}_{ot}")
                     for ot in range(N_OT)]
            phibs = {}
            for t2 in range(N_T2):
                # ---- phi production for the 2 i-chunks of this tile ----
                phi8 = phi8_pool.tile([P, 2 * N_PAIR, B_SUPER], F8, tag="phi8")
                for sub in range(2):
                    ichk = 2 * t2 + sub
                    if bs == 0 and ichk == 0:
                        x_sb = x_first
                    else:
                        x_sb = x_pool.tile([P, B_SUPER], F32, tag="xin")
                        nc.scalar.dma_start(x_sb[:], xt[bs, ichk])
                    base = sub * 9
                    for j in range(8):
                        t = t_pool.tile([P, B_SUPER], F32, tag="tmp")
                        nc.scalar.activation(t[:], x_sb[:], AF.Relu,
                                             bias=consts[:, j:j + 1],
                                             scale=-2.0)
                        nc.vector.tensor_scalar(phi8[:, base + j, :], t[:],
                                                1.0, -0.5, ALU.min, ALU.add)
                    # xc channel: clip(x,-2,2)/4 = clip(x/4, -1/2, 1/2)
                    t2c = t_pool.tile([P, B_SUPER], F32, tag="tmp")
                    nc.vector.tensor_scalar(t2c[:], x_sb[:], 1.0 / XCS,
                                            2.0 / XCS, ALU.mult, ALU.min)
                    nc.vector.tensor_scalar_max(phi8[:, base + 8, :], t2c[:],
                                                -2.0 / XCS)
                    # silu channel (bf16)
                    phib = phib_pool.tile([P, B_SUPER], BF16, tag="phib",
                                          name=f"phib_{bs}_{ichk}")
                    nc.scalar.activation(phib[:], x_sb[:], AF.Silu)
                    phibs[ichk] = phib

                # ---- matmuls ----
                last_tile = (t2 == N_T2 - 1)
                if last_tile:
                    # finish banks one at a time so evacuation and output
                    # DMA overlap the remaining matmuls
                    for ot in range(N_OT):
                        for tp in range(N_PAIR):
                            nc.tensor.matmul(
                                psums[ot][:],
                                lhsT=wr_sb[:, t2 * N_PAIR + tp, :,
                                           bass.ts(ot, P)],
                                rhs=phi8[:, 2 * tp:2 * tp + 2, :],
                                perf_mode=DR, start=False, stop=False)
                        for sub in range(2):
                            nc.tensor.matmul(
                                psums[ot][:],
                                lhsT=wb_sb[:, 2 * t2 + sub, bass.ts(ot, P)],
                                rhs=phibs[2 * t2 + sub][:],
                                start=False, stop=(sub == 1))
                else:
                    for tp in range(N_PAIR):
                        for ot in range(N_OT):
                            nc.tensor.matmul(
                                psums[ot][:],
                                lhsT=wr_sb[:, t2 * N_PAIR + tp, :,
                                           bass.ts(ot, P)],
                                rhs=phi8[:, 2 * tp:2 * tp + 2, :],
                                perf_mode=DR,
                                start=(t2 == 0 and tp == 0), stop=False)
                    for sub in range(2):
                        for ot in range(N_OT):
                            nc.tensor.matmul(
                                psums[ot][:],
                                lhsT=wb_sb[:, 2 * t2 + sub, bass.ts(ot, P)],
                                rhs=phibs[2 * t2 + sub][:],
                                start=False, stop=False)

            # evacuate PSUM: out = psum/ALPHA + bias (DVE) into one wide
            # tile, then a single 2MB store (DGE stripes it across rings)
            o_sb = out_pool.tile([P, N_OT, B_SUPER], F32, tag="outs")
            for ot in range(N_OT):
                nc.vector.tensor_scalar(o_sb[:, ot, :], psums[ot][:],
                                        1.0 / ALPHA, bias_sb[:, ot:ot + 1],
                                        ALU.mult, ALU.add)
            nc.gpsimd.dma_start(outT[bs], o_sb[:])

    nc.compile()
    return nc


def _get_nc():
    if "nc" not in _CACHE:
        _CACHE["nc"] = _build()
    return _CACHE["nc"]


def _quantize_ramps(rho, x):
    """Exact 256-path fp8 quantization of ramp weights.

    rho: [O, I, 8] fp64 unscaled ramp weights (sw_j - sw_{j+1}).
    x:   [B, I] fp32 inputs (defines the Gram weighting).

    Returns (q_scaled [O,I,8] e4m3-representable fp32 = approx rho*ALPHA,
             lam [O,I] clip-channel weight, mu [O,I] silu-weight correction,
             dbias [O] bias correction).
    """
    grid = np.linspace(-2.0, 2.0, 9).astype(np.float64)
    # moments under the empirical x distribution (subsampled)
    xr = x.reshape(-1)[::16].astype(np.float64)
    xs = np.clip(xr, -2.0, 2.0)
    H = np.maximum(0.0, 1.0 - np.abs((xs[:, None] - grid) / 0.5))  # [N, 9]
    silu = xr / (1.0 + np.exp(-xr))
    Psi = np.stack([np.ones_like(xr), xs, silu], axis=1)     # [N, 3]
    n = xr.size
    G = (H.T @ H) / n                                        # [9, 9]
    M = (Psi.T @ Psi) / n                                    # [3, 3]
    K = (Psi.T @ H) / n                                      # [3, 9]
    Minv = np.linalg.inv(M)
    Gt = G - K.T @ Minv @ K                                  # residual metric
    # C[j,k] = sum_{m>j, n>k} Gt[m,n]  (P_m = sum_{j<m} err_j)
    Csuf = np.cumsum(np.cumsum(Gt[::-1, ::-1], 0), 1)[::-1, ::-1]
    C = np.ascontiguousarray(Csuf[1:, 1:]).astype(np.float32)

    v = (rho * ALPHA).reshape(-1, 8).astype(np.float32)      # [R, 8]
    q0 = v.astype(E4M3)
    q0f = q0.astype(np.float32)
    up = np.nextafter(q0, np.array(np.inf, dtype=E4M3)).astype(np.float32)
    dn = np.nextafter(q0, np.array(-np.inf, dtype=E4M3)).astype(np.float32)
    lo = np.where(q0f <= v, q0f, dn)
    hi = np.where(q0f >= v, q0f, up)
    e_lo = lo - v
    d = hi - lo                                              # >= 0

    S = ((np.arange(256)[:, None] >> np.arange(8)[None, :]) & 1
         ).astype(np.float32)                                # [256, 8]
    SS = (S[:, :, None] * S[:, None, :]).reshape(256, 64)    # [256, 64]

    R = v.shape[0]
    q = np.empty_like(v)
    CHR = 131072
    for s in range(0, R, CHR):
        el, dd = e_lo[s:s + CHR], d[s:s + CHR]
        u = el @ C                                           # [r, 8]
        lin = 2.0 * dd * u                                   # [r, 8]
        V = (dd[:, :, None] * dd[:, None, :] * C[None]).reshape(-1, 64)
        Q = lin @ S.T + V @ SS.T                             # [r, 256]
        best = Q.argmin(axis=1)
        sel = S[best]                                        # [r, 8]
        q[s:s + CHR] = lo[s:s + CHR] + sel * dd

    q_oi = q.reshape(O, I, 8).astype(np.float64)
    eps = rho - q_oi / ALPHA                                 # unscaled error
    Pm = np.zeros((O, I, 9))
    Pm[..., 1:] = np.cumsum(eps, axis=-1)
    a = np.einsum('rm,oim->oir', Minv @ K, Pm)               # [O, I, 3]
    c, lam, mu = a[..., 0], a[..., 1], a[..., 2]
    T = Pm[..., 8]
    dbias = (0.5 * T - c).sum(axis=1)                        # [O]
    return q.reshape(O, I, 8), lam, mu, dbias


def _stage_inputs(x, spline_weight, base_weight, bias):
    """Host-side input staging shared by kernel() and test harnesses."""
    # x[b, i] -> [core, bs, ichk, p, b_super]
    xt = np.ascontiguousarray(
        x.reshape(N_CORES, N_SUPERS, B_SUPER, N_ICHK, P)
        .transpose(0, 1, 3, 4, 2))

    sw = spline_weight.astype(np.float64)
    rho = sw[..., :8] - sw[..., 1:]                          # [O, I, 8]
    q_scaled, lam, mu, dbias = _quantize_ramps(rho, x)

    # channel 9 of each i-chunk: xc weights = -ALPHA*XCS*lam, e4m3
    xcw = (-ALPHA * XCS * lam).astype(np.float32).astype(E4M3) \
        .astype(np.float32)
    w9 = np.concatenate([q_scaled, xcw[..., None]], axis=2)  # [O, I, 9]
    # wr[p, t2*9+tp, r, o] = w9[o, (2*t2+cc//9)*128+p, cc%9], cc = 2*tp+r
    wr_dev = np.ascontiguousarray(
        w9.reshape(O, N_ICHK, P, 9)
        .transpose(1, 3, 2, 0)                               # [ichk, ch, p, o]
        .reshape(N_T2, 2 * 9, P, O)                          # [t2, cc, p, o]
        .reshape(N_T2, N_PAIR, 2, P, O)
        .transpose(3, 0, 1, 2, 4)                            # [p, t2, tp, r, o]
        .reshape(P, N_T2 * N_PAIR, 2, O).astype(E4M3))

    # wb[p, ichk, o] = 32*(bw - mu)[o, i]
    wb_dev = np.ascontiguousarray(
        (ALPHA * (base_weight.astype(np.float64) - mu))
        .T.reshape(N_ICHK, P, O)
        .transpose(1, 0, 2).astype(NP_BF16))                 # [P, ichk, O]

    # bias fold: bias + 0.5*sum_i(sw0 + sw8) + dbias
    bias_dev = (bias.astype(np.float64)
                + 0.5 * (sw[..., 0].sum(axis=1) + sw[..., 8].sum(axis=1))
                + dbias).astype(np.float32)
    return xt, wr_dev, wb_dev, bias_dev


def kernel(x, spline_weight, base_weight, bias):
    x = np.asarray(x, dtype=np.float32)
    spline_weight = np.asarray(spline_weight, dtype=np.float32)
    base_weight = np.asarray(base_weight, dtype=np.float32)
    bias = np.asarray(bias, dtype=np.float32)

    nc = _get_nc()
    xt, wr_dev, wb_dev, bias_dev = _stage_inputs(
        x, spline_weight, base_weight, bias)

    in_maps = [{"xt": np.ascontiguousarray(xt[c]), "wr": wr_dev,
                "wb": wb_dev, "bias": bias_dev} for c in range(N_CORES)]
    res = run_bass_kernel_spmd(nc, in_maps, core_ids=list(range(N_CORES)))

    # outT[bs, p, ot, b] per core -> out[b, o]
    outs = []
    for c in range(N_CORES):
        oc = np.asarray(res.results[c]["outT"])
        outs.append(oc.transpose(0, 3, 2, 1).reshape(B_CORE, O))
    return np.ascontiguousarray(np.concatenate(outs, axis=0),
                                dtype=np.float32)


# revision 27
# speedup vs baseline: 1.0212x; 1.0212x over previous
"""KAN basis-linear kernel for 8 TRN2 NeuronCores — fp8 DoubleRow edition.

Computes, for x:[B,I], spline_weight:[O,I,K=9], base_weight:[O,I], bias:[O]:

    basis = relu(1 - |(clip(x,-2,2)[...,None] - grid) / delta|)   # hat basis
    out   = einsum('bik,oik->bo', basis, spline_weight)
          + silu(x) @ base_weight.T + bias

Data-parallel over the batch across 8 cores (weights replicated).

Algebra: with grid g_k = -2 + 0.5k, Abel summation turns the hat-basis
contraction into 8 saturating-ramp channels psi_j = clip(2(g_{j+1}-x),0,1);
shifting to phi'_j = psi_j - 0.5 (saturated values +-0.5, exactly
representable in fp8) folds a constant into the bias. The 8 ramp channels
plus a clip(x)/4 error-absorber channel run as fp8(e4m3) DoubleRow matmuls
(2 contraction rows/cycle = 2x bf16 rate), pairing the 18 channels of each
two-i-chunk phi tile into 9 pairs; the silu base branch stays bf16.

fp8 weight quantization is chosen per (o,i) by exact 256-path search: the
output error equals sum_i [P_i(x) - absorber_i(x)] where P_i linearly
interpolates the cumulative quantization errors over the 8 knots; each
(o,i) row has 2^8 floor/ceil choices, scored by a Gram-metric quadratic
form under the empirical x distribution. The best absorber component over
{1, clip(x), silu(x)} is folded into the bias, the clip(x) channel weights
(lambda), and the silu channel weights (mu) — the latter two for free.
"""
import numpy as np
import ml_dtypes
from contextlib import ExitStack

import concourse.bass as bass
import concourse.tile as tile
import concourse.mybir as mybir
from concourse import bacc
from concourse.bass_utils import run_bass_kernel_spmd

N_CORES = 8
B, I, O = 16384, 1024, 1024
B_CORE = B // N_CORES            # 2048 batch rows per core
B_SUPER = 512                    # batch stripe held in PSUM (1 bank per o-tile)
N_SUPERS = B_CORE // B_SUPER     # 4
P = 128
N_ICHK = 8                       # contraction chunks over i
N_T2 = 4                         # phi tiles per stripe (2 i-chunks each)
N_PAIR = 9                       # DoubleRow pairs per 2-ichk tile (18 channels)
N_OT = O // P                    # 8 output tiles (one PSUM bank each)
ALPHA = 32.0                     # fp8 weight scale (psum divided back on evac)
XCS = 4.0                        # clip(x) channel prescale: phi_xc = xc/XCS

F32 = mybir.dt.float32
BF16 = mybir.dt.bfloat16
F8 = mybir.dt.float8e4
E4M3 = ml_dtypes.float8_e4m3
NP_BF16 = ml_dtypes.bfloat16
AF = mybir.ActivationFunctionType
ALU = mybir.AluOpType
DR = mybir.MatmulPerfMode.DoubleRow

_CACHE = {}


def _build():
    nc = bacc.Bacc("TRN2", target_bir_lowering=False, debug=False,
                   num_devices=N_CORES)
    # x tiled on host: [bs, ichk, p, b]
    xt = nc.dram_tensor("xt", [N_SUPERS, N_ICHK, P, B_SUPER], F32,
                        kind="ExternalInput").ap()
    # fp8 weights: [p, 36 pairs, 2, o]; pair t2*9+tp slot r is channel
    # cc=2*tp+r of tile t2: (ichk=2*t2+cc//9, ch=cc%9), ch 0-7 ramps, 8 xc
    wr = nc.dram_tensor("wr", [P, N_T2 * N_PAIR, 2, O], F8,
                        kind="ExternalInput").ap()
    # bf16 silu weights: [p, ichk, o]
    wb = nc.dram_tensor("wb", [P, N_ICHK, O], BF16,
                        kind="ExternalInput").ap()
    bias = nc.dram_tensor("bias", [O], F32, kind="ExternalInput").ap()
    # output tiled: [ot, bs, p, b] (contiguous 256KB stores)
    outT = nc.dram_tensor("outT", [N_OT, N_SUPERS, P, B_SUPER], F32,
                          kind="ExternalOutput").ap()

    with tile.TileContext(nc) as tc, ExitStack() as ctx:
        const_pool = ctx.enter_context(tc.tile_pool(name="const", bufs=1))
        x_pool = ctx.enter_context(tc.tile_pool(name="xin", bufs=3))
        t_pool = ctx.enter_context(tc.tile_pool(name="tmp", bufs=4))
        phi8_pool = ctx.enter_context(tc.tile_pool(name="phi8", bufs=3))
        phib_pool = ctx.enter_context(tc.tile_pool(name="phib", bufs=4))
        w_pool = ctx.enter_context(tc.tile_pool(name="wts", bufs=1))
        out_pool = ctx.enter_context(tc.tile_pool(name="outs", bufs=3))
        psum_pool = ctx.enter_context(
            tc.tile_pool(name="psum", bufs=N_OT, space="PSUM"))

        # first x chunk ASAP (phi production is the kernel's lead-in)
        x_first = x_pool.tile([P, B_SUPER], F32, tag="xin")
        nc.scalar.dma_start(x_first[:], xt[0, 0])

        # ACT bias constants: 2*g_{j+1} = j - 3 for j=0..7
        consts = const_pool.tile([P, 8], F32)
        for j in range(8):
            nc.any.memset(consts[:, j:j + 1], float(j - 3))

        # bias[o] -> [128, 8] with o = ot*128 + p
        bias_sb = const_pool.tile([P, N_OT], F32)
        nc.scalar.dma_start(bias_sb[:], bias.rearrange("(ot p) -> p ot", p=P))

        # resident weights, streamed in consumption order on the sync queue
        wr_sb = w_pool.tile([P, N_T2 * N_PAIR, 2, O], F8)
        wb_sb = w_pool.tile([P, N_ICHK, O], BF16)
        for tp in range(N_PAIR):  # first tile per-pair so matmul 0 starts early
            nc.sync.dma_start(wr_sb[:, tp:tp + 1], wr[:, tp:tp + 1])
        nc.sync.dma_start(wb_sb[:, 0:2], wb[:, 0:2])
        for t2 in range(1, N_T2):
            nc.sync.dma_start(wr_sb[:, t2 * N_PAIR:(t2 + 1) * N_PAIR],
                              wr[:, t2 * N_PAIR:(t2 + 1) * N_PAIR])
            nc.sync.dma_start(wb_sb[:, 2 * t2:2 * t2 + 2], wb[:, 2 * t2:2 * t2 + 2])


        # Small PE warm-up spin bridging the first input-DMA wait: starts
        # the HAM busy-streak early so the clock-gate reaches 8/8 sooner.
        warm = const_pool.tile([P, B_SUPER], BF16)
        nc.any.memset(warm[:], 0.0)
        warm_ps = psum_pool.tile([P, B_SUPER], F32, tag="psum")
        for _ in range(8):
            nc.tensor.matmul(warm_ps[:], lhsT=warm[:, :P], rhs=warm[:],
                             start=True, stop=True)

        for bs in range(N_SUPERS):
            psums = [psum_pool.tile([P, B_SUPER], F32, tag="psum",
                                    name=f"psum_{bs}_{ot}")
                     for ot in range(N_OT)]
            phibs = {}
            for t2 in range(N_T2):
                # ---- phi production for the 2 i-chunks of this tile ----
                phi8 = phi8_pool.tile([P, 2 * N_PAIR, B_SUPER], F8, tag="phi8")
                for sub in range(2):
                    ichk = 2 * t2 + sub
                    if bs == 0 and ichk == 0:
                        x_sb = x_first
                    else:
                        x_sb = x_pool.tile([P, B_SUPER], F32, tag="xin")
                        nc.scalar.dma_start(x_sb[:], xt[bs, ichk])
                    base = sub * 9
                    for j in range(8):
                        t = t_pool.tile([P, B_SUPER], F32, tag="tmp")
                        nc.scalar.activation(t[:], x_sb[:], AF.Relu,
                                             bias=consts[:, j:j + 1],
                                             scale=-2.0)
                        nc.vector.tensor_scalar(phi8[:, base + j, :], t[:],
                                                1.0, -0.5, ALU.min, ALU.add)
                    # xc channel: clip(x,-2,2)/4 = clip(x/4, -1/2, 1/2)
                    t2c = t_pool.tile([P, B_SUPER], F32, tag="tmp")
                    nc.vector.tensor_scalar(t2c[:], x_sb[:], 1.0 / XCS,
                                            2.0 / XCS, ALU.mult, ALU.min)
                    nc.vector.tensor_scalar_max(phi8[:, base + 8, :], t2c[:],
                                                -2.0 / XCS)
                    # silu channel (bf16)
                    phib = phib_pool.tile([P, B_SUPER], BF16, tag="phib",
                                          name=f"phib_{bs}_{ichk}")
                    nc.scalar.activation(phib[:], x_sb[:], AF.Silu)
                    phibs[ichk] = phib

                # ---- matmuls ----
                last_tile = (t2 == N_T2 - 1)
                if last_tile:
                    # finish banks one at a time so evacuation and output
                    # DMA overlap the remaining matmuls
                    for ot in range(N_OT):
                        for tp in range(N_PAIR):
                            nc.tensor.matmul(
                                psums[ot][:],
                                lhsT=wr_sb[:, t2 * N_PAIR + tp, :,
                                           bass.ts(ot, P)],
                                rhs=phi8[:, 2 * tp:2 * tp + 2, :],
                                perf_mode=DR, start=False, stop=False)
                        for sub in range(2):
                            nc.tensor.matmul(
                                psums[ot][:],
                                lhsT=wb_sb[:, 2 * t2 + sub, bass.ts(ot, P)],
                                rhs=phibs[2 * t2 + sub][:],
                                start=False, stop=(sub == 1))
                else:
                    for tp in range(N_PAIR):
                        for ot in range(N_OT):
                            nc.tensor.matmul(
                                psums[ot][:],
                                lhsT=wr_sb[:, t2 * N_PAIR + tp, :,
                                           bass.ts(ot, P)],
                                rhs=phi8[:, 2 * tp:2 * tp + 2, :],
                                perf_mode=DR,
                                start=(t2 == 0 and tp == 0), stop=False)
                    for sub in range(2):
                        for ot in range(N_OT):
                            nc.tensor.matmul(
                                psums[ot][:],
                                lhsT=wb_sb[:, 2 * t2 + sub, bass.ts(ot, P)],
                                rhs=phibs[2 * t2 + sub][:],
                                start=False, stop=False)

            # evacuate PSUM: out = psum/ALPHA + bias (DVE), stores alternate
            # between two queues so the final ones don't serialize
            for ot in range(N_OT):
                o_sb = out_pool.tile([P, B_SUPER], F32, tag="outs")
                nc.vector.tensor_scalar(o_sb[:], psums[ot][:], 1.0 / ALPHA,
                                        bias_sb[:, ot:ot + 1],
                                        ALU.mult, ALU.add)
                nc.gpsimd.dma_start(outT[ot, bs], o_sb[:])

    nc.compile()
    return nc


def _get_nc():
    if "nc" not in _CACHE:
        _CACHE["nc"] = _build()
    return _CACHE["nc"]


def _quantize_ramps(rho, x):
    """Exact 256-path fp8 quantization of ramp weights.

    rho: [O, I, 8] fp64 unscaled ramp weights (sw_j - sw_{j+1}).
    x:   [B, I] fp32 inputs (defines the Gram weighting).

    Returns (q_scaled [O,I,8] e4m3-representable fp32 = approx rho*ALPHA,
             lam [O,I] clip-channel weight, mu [O,I] silu-weight correction,
             dbias [O] bias correction).
    """
    grid = np.linspace(-2.0, 2.0, 9).astype(np.float64)
    # moments under the empirical x distribution (subsampled)
    xr = x.reshape(-1)[::16].astype(np.float64)
    xs = np.clip(xr, -2.0, 2.0)
    H = np.maximum(0.0, 1.0 - np.abs((xs[:, None] - grid) / 0.5))  # [N, 9]
    silu = xr / (1.0 + np.exp(-xr))
    Psi = np.stack([np.ones_like(xr), xs, silu], axis=1)     # [N, 3]
    n = xr.size
    G = (H.T @ H) / n                                        # [9, 9]
    M = (Psi.T @ Psi) / n                                    # [3, 3]
    K = (Psi.T @ H) / n                                      # [3, 9]
    Minv = np.linalg.inv(M)
    Gt = G - K.T @ Minv @ K                                  # residual metric
    # C[j,k] = sum_{m>j, n>k} Gt[m,n]  (P_m = sum_{j<m} err_j)
    Csuf = np.cumsum(np.cumsum(Gt[::-1, ::-1], 0), 1)[::-1, ::-1]
    C = np.ascontiguousarray(Csuf[1:, 1:]).astype(np.float32)

    v = (rho * ALPHA).reshape(-1, 8).astype(np.float32)      # [R, 8]
    q0 = v.astype(E4M3)
    q0f = q0.astype(np.float32)
    up = np.nextafter(q0, np.array(np.inf, dtype=E4M3)).astype(np.float32)
    dn = np.nextafter(q0, np.array(-np.inf, dtype=E4M3)).astype(np.float32)
    lo = np.where(q0f <= v, q0f, dn)
    hi = np.where(q0f >= v, q0f, up)
    e_lo = lo - v
    d = hi - lo                                              # >= 0

    S = ((np.arange(256)[:, None] >> np.arange(8)[None, :]) & 1
         ).astype(np.float32)                                # [256, 8]
    SS = (S[:, :, None] * S[:, None, :]).reshape(256, 64)    # [256, 64]

    R = v.shape[0]
    q = np.empty_like(v)
    CHR = 131072
    for s in range(0, R, CHR):
        el, dd = e_lo[s:s + CHR], d[s:s + CHR]
        u = el @ C                                           # [r, 8]
        lin = 2.0 * dd * u                                   # [r, 8]
        V = (dd[:, :, None] * dd[:, None, :] * C[None]).reshape(-1, 64)
        Q = lin @ S.T + V @ SS.T                             # [r, 256]
        best = Q.argmin(axis=1)
        sel = S[best]                                        # [r, 8]
        q[s:s + CHR] = lo[s:s + CHR] + sel * dd

    q_oi = q.reshape(O, I, 8).astype(np.float64)
    eps = rho - q_oi / ALPHA                                 # unscaled error
    Pm = np.zeros((O, I, 9))
    Pm[..., 1:] = np.cumsum(eps, axis=-1)
    a = np.einsum('rm,oim->oir', Minv @ K, Pm)               # [O, I, 3]
    c, lam, mu = a[..., 0], a[..., 1], a[..., 2]
    T = Pm[..., 8]
    dbias = (0.5 * T - c).sum(axis=1)                        # [O]
    return q.reshape(O, I, 8), lam, mu, dbias


def _stage_inputs(x, spline_weight, base_weight, bias):
    """Host-side input staging shared by kernel() and test harnesses."""
    # x[b, i] -> [core, bs, ichk, p, b_super]
    xt = np.ascontiguousarray(
        x.reshape(N_CORES, N_SUPERS, B_SUPER, N_ICHK, P)
        .transpose(0, 1, 3, 4, 2))

    sw = spline_weight.astype(np.float64)
    rho = sw[..., :8] - sw[..., 1:]                          # [O, I, 8]
    q_scaled, lam, mu, dbias = _quantize_ramps(rho, x)

    # channel 9 of each i-chunk: xc weights = -ALPHA*XCS*lam, e4m3
    xcw = (-ALPHA * XCS * lam).astype(np.float32).astype(E4M3) \
        .astype(np.float32)
    w9 = np.concatenate([q_scaled, xcw[..., None]], axis=2)  # [O, I, 9]
    # wr[p, t2*9+tp, r, o] = w9[o, (2*t2+cc//9)*128+p, cc%9], cc = 2*tp+r
    wr_dev = np.ascontiguousarray(
        w9.reshape(O, N_ICHK, P, 9)
        .transpose(1, 3, 2, 0)                               # [ichk, ch, p, o]
        .reshape(N_T2, 2 * 9, P, O)                          # [t2, cc, p, o]
        .reshape(N_T2, N_PAIR, 2, P, O)
        .transpose(3, 0, 1, 2, 4)                            # [p, t2, tp, r, o]
        .reshape(P, N_T2 * N_PAIR, 2, O).astype(E4M3))

    # wb[p, ichk, o] = 32*(bw - mu)[o, i]
    wb_dev = np.ascontiguousarray(
        (ALPHA * (base_weight.astype(np.float64) - mu))
        .T.reshape(N_ICHK, P, O)
        .transpose(1, 0, 2).astype(NP_BF16))                 # [P, ichk, O]

    # bias fold: bias + 0.5*sum_i(sw0 + sw8) + dbias
    bias_dev = (bias.astype(np.float64)
                + 0.5 * (sw[..., 0].sum(axis=1) + sw[..., 8].sum(axis=1))
                + dbias).astype(np.float32)
    return xt, wr_dev, wb_dev, bias_dev


def kernel(x, spline_weight, base_weight, bias):
    x = np.asarray(x, dtype=np.float32)
    spline_weight = np.asarray(spline_weight, dtype=np.float32)
    base_weight = np.asarray(base_weight, dtype=np.float32)
    bias = np.asarray(bias, dtype=np.float32)

    nc = _get_nc()
    xt, wr_dev, wb_dev, bias_dev = _stage_inputs(
        x, spline_weight, base_weight, bias)

    in_maps = [{"xt": np.ascontiguousarray(xt[c]), "wr": wr_dev,
                "wb": wb_dev, "bias": bias_dev} for c in range(N_CORES)]
    res = run_bass_kernel_spmd(nc, in_maps, core_ids=list(range(N_CORES)))

    # outT[ot, bs, p, b] per core -> out[b, o]
    outs = []
    for c in range(N_CORES):
        oc = np.asarray(res.results[c]["outT"])
        outs.append(oc.transpose(1, 3, 0, 2).reshape(B_CORE, O))
    return np.ascontiguousarray(np.concatenate(outs, axis=0),
                                dtype=np.float32)


# revision 29
# speedup vs baseline: 1.0285x; 1.0072x over previous
"""KAN basis-linear kernel for 8 TRN2 NeuronCores — fp8 DoubleRow edition.

Computes, for x:[B,I], spline_weight:[O,I,K=9], base_weight:[O,I], bias:[O]:

    basis = relu(1 - |(clip(x,-2,2)[...,None] - grid) / delta|)   # hat basis
    out   = einsum('bik,oik->bo', basis, spline_weight)
          + silu(x) @ base_weight.T + bias

Data-parallel over the batch across 8 cores (weights replicated).

Algebra: with grid g_k = -2 + 0.5k, Abel summation turns the hat-basis
contraction into 8 saturating-ramp channels psi_j = clip(2(g_{j+1}-x),0,1);
shifting to phi'_j = psi_j - 0.5 (saturated values +-0.5, exactly
representable in fp8) folds a constant into the bias. The 8 ramp channels
plus a clip(x)/4 error-absorber channel run as fp8(e4m3) DoubleRow matmuls
(2 contraction rows/cycle = 2x bf16 rate), pairing the 18 channels of each
two-i-chunk phi tile into 9 pairs; the silu base branch stays bf16.

fp8 weight quantization is chosen per (o,i) by exact 256-path search: the
output error equals sum_i [P_i(x) - absorber_i(x)] where P_i linearly
interpolates the cumulative quantization errors over the 8 knots; each
(o,i) row has 2^8 floor/ceil choices, scored by a Gram-metric quadratic
form under the empirical x distribution. The best absorber component over
{1, clip(x), silu(x)} is folded into the bias, the clip(x) channel weights
(lambda), and the silu channel weights (mu) — the latter two for free.
"""
import numpy as np
import ml_dtypes
from contextlib import ExitStack

import concourse.bass as bass
import concourse.tile as tile
import concourse.mybir as mybir
from concourse import bacc
from concourse.bass_utils import run_bass_kernel_spmd

N_CORES = 8
B, I, O = 16384, 1024, 1024
B_CORE = B // N_CORES            # 2048 batch rows per core
B_SUPER = 512                    # batch stripe held in PSUM (1 bank per o-tile)
N_SUPERS = B_CORE // B_SUPER     # 4
P = 128
N_ICHK = 8                       # contraction chunks over i
N_T2 = 4                         # phi tiles per stripe (2 i-chunks each)
N_PAIR = 9                       # DoubleRow pairs per 2-ichk tile (18 channels)
N_OT = O // P                    # 8 output tiles (one PSUM bank each)
ALPHA = 32.0                     # fp8 weight scale (psum divided back on evac)
XCS = 4.0                        # clip(x) channel prescale: phi_xc = xc/XCS

F32 = mybir.dt.float32
BF16 = mybir.dt.bfloat16
F8 = mybir.dt.float8e4
E4M3 = ml_dtypes.float8_e4m3
NP_BF16 = ml_dtypes.bfloat16
AF = mybir.ActivationFunctionType
ALU = mybir.AluOpType
DR = mybir.MatmulPerfMode.DoubleRow

_CACHE = {}


def _build():
    nc = bacc.Bacc("TRN2", target_bir_lowering=False, debug=False,
                   num_devices=N_CORES)
    # x tiled on host: [bs, ichk, p, b]
    xt = nc.dram_tensor("xt", [N_SUPERS, N_ICHK, P, B_SUPER], F32,
                        kind="ExternalInput").ap()
    # fp8 weights: [p, 36 pairs, 2, o]; pair t2*9+tp slot r is channel
    # cc=2*tp+r of tile t2: (ichk=2*t2+cc//9, ch=cc%9), ch 0-7 ramps, 8 xc
    wr = nc.dram_tensor("wr", [P, N_T2 * N_PAIR, 2, O], F8,
                        kind="ExternalInput").ap()
    # bf16 silu weights: [p, ichk, o]
    wb = nc.dram_tensor("wb", [P, N_ICHK, O], BF16,
                        kind="ExternalInput").ap()
    bias = nc.dram_tensor("bias", [O], F32, kind="ExternalInput").ap()
    # output tiled: [ot, bs, p, b] (contiguous 256KB stores)
    outT = nc.dram_tensor("outT", [N_OT, N_SUPERS, P, B_SUPER], F32,
                          kind="ExternalOutput").ap()

    with tile.TileContext(nc) as tc, ExitStack() as ctx:
        const_pool = ctx.enter_context(tc.tile_pool(name="const", bufs=1))
        x_pool = ctx.enter_context(tc.tile_pool(name="xin", bufs=3))
        t_pool = ctx.enter_context(tc.tile_pool(name="tmp", bufs=4))
        phi8_pool = ctx.enter_context(tc.tile_pool(name="phi8", bufs=3))
        phib_pool = ctx.enter_context(tc.tile_pool(name="phib", bufs=4))
        w_pool = ctx.enter_context(tc.tile_pool(name="wts", bufs=1))
        out_pool = ctx.enter_context(tc.tile_pool(name="outs", bufs=3))
        psum_pool = ctx.enter_context(
            tc.tile_pool(name="psum", bufs=N_OT, space="PSUM"))

        # first x chunk ASAP (phi production is the kernel's lead-in)
        x_first = x_pool.tile([P, B_SUPER], F32, tag="xin")
        nc.scalar.dma_start(x_first[:], xt[0, 0])

        # ACT bias constants: 2*g_{j+1} = j - 3 for j=0..7
        consts = const_pool.tile([P, 8], F32)
        for j in range(8):
            nc.any.memset(consts[:, j:j + 1], float(j - 3))

        # bias[o] -> [128, 8] with o = ot*128 + p
        bias_sb = const_pool.tile([P, N_OT], F32)
        nc.scalar.dma_start(bias_sb[:], bias.rearrange("(ot p) -> p ot", p=P))

        # resident weights, streamed in consumption order on the sync queue
        wr_sb = w_pool.tile([P, N_T2 * N_PAIR, 2, O], F8)
        wb_sb = w_pool.tile([P, N_ICHK, O], BF16)
        for tp in range(N_PAIR):  # first tile per-pair so matmul 0 starts early
            nc.sync.dma_start(wr_sb[:, tp:tp + 1], wr[:, tp:tp + 1])
        nc.sync.dma_start(wb_sb[:, 0:2], wb[:, 0:2])
        for t2 in range(1, N_T2):
            nc.sync.dma_start(wr_sb[:, t2 * N_PAIR:(t2 + 1) * N_PAIR],
                              wr[:, t2 * N_PAIR:(t2 + 1) * N_PAIR])
            nc.sync.dma_start(wb_sb[:, 2 * t2:2 * t2 + 2], wb[:, 2 * t2:2 * t2 + 2])


        # Small PE warm-up spin bridging the first input-DMA wait: starts
        # the HAM busy-streak early so the clock-gate reaches 8/8 sooner.
        warm = const_pool.tile([P, B_SUPER], BF16)
        nc.any.memset(warm[:], 0.0)
        warm_ps = psum_pool.tile([P, B_SUPER], F32, tag="psum")
        for _ in range(11):
            nc.tensor.matmul(warm_ps[:], lhsT=warm[:, :P], rhs=warm[:],
                             start=True, stop=True)

        for bs in range(N_SUPERS):
            psums = [psum_pool.tile([P, B_SUPER], F32, tag="psum",
                                    name=f"psum_{bs}_{ot}")
                     for ot in range(N_OT)]
            phibs = {}
            for t2 in range(N_T2):
                # ---- phi production for the 2 i-chunks of this tile ----
                phi8 = phi8_pool.tile([P, 2 * N_PAIR, B_SUPER], F8, tag="phi8")
                for sub in range(2):
                    ichk = 2 * t2 + sub
                    if bs == 0 and ichk == 0:
                        x_sb = x_first
                    else:
                        x_sb = x_pool.tile([P, B_SUPER], F32, tag="xin")
                        nc.scalar.dma_start(x_sb[:], xt[bs, ichk])
                    base = sub * 9
                    for j in range(8):
                        t = t_pool.tile([P, B_SUPER], F32, tag="tmp")
                        nc.scalar.activation(t[:], x_sb[:], AF.Relu,
                                             bias=consts[:, j:j + 1],
                                             scale=-2.0)
                        nc.vector.tensor_scalar(phi8[:, base + j, :], t[:],
                                                1.0, -0.5, ALU.min, ALU.add)
                    # xc channel: clip(x,-2,2)/4 = clip(x/4, -1/2, 1/2)
                    t2c = t_pool.tile([P, B_SUPER], F32, tag="tmp")
                    nc.vector.tensor_scalar(t2c[:], x_sb[:], 1.0 / XCS,
                                            2.0 / XCS, ALU.mult, ALU.min)
                    nc.vector.tensor_scalar_max(phi8[:, base + 8, :], t2c[:],
                                                -2.0 / XCS)
                    # silu channel (bf16)
                    phib = phib_pool.tile([P, B_SUPER], BF16, tag="phib",
                                          name=f"phib_{bs}_{ichk}")
                    nc.scalar.activation(phib[:], x_sb[:], AF.Silu)
                    phibs[ichk] = phib

                # ---- matmuls ----
                last_tile = (t2 == N_T2 - 1)
                if last_tile:
                    # finish banks one at a time so evacuation and output
                    # DMA overlap the remaining matmuls
                    for ot in range(N_OT):
                        for tp in range(N_PAIR):
                            nc.tensor.matmul(
                                psums[ot][:],
                                lhsT=wr_sb[:, t2 * N_PAIR + tp, :,
                                           bass.ts(ot, P)],
                                rhs=phi8[:, 2 * tp:2 * tp + 2, :],
                                perf_mode=DR, start=False, stop=False)
                        for sub in range(2):
                            nc.tensor.matmul(
                                psums[ot][:],
                                lhsT=wb_sb[:, 2 * t2 + sub, bass.ts(ot, P)],
                                rhs=phibs[2 * t2 + sub][:],
                                start=False, stop=(sub == 1))
                else:
                    for tp in range(N_PAIR):
                        for ot in range(N_OT):
                            nc.tensor.matmul(
                                psums[ot][:],
                                lhsT=wr_sb[:, t2 * N_PAIR + tp, :,
                                           bass.ts(ot, P)],
                                rhs=phi8[:, 2 * tp:2 * tp + 2, :],
                                perf_mode=DR,
                                start=(t2 == 0 and tp == 0), stop=False)
                    for sub in range(2):
                        for ot in range(N_OT):
                            nc.tensor.matmul(
                                psums[ot][:],
                                lhsT=wb_sb[:, 2 * t2 + sub, bass.ts(ot, P)],
                                rhs=phibs[2 * t2 + sub][:],
                                start=False, stop=False)

            # evacuate PSUM: out = psum/ALPHA + bias (DVE). The last stripe's
            # stores alternate across two queues: each dma_start carries ~1us
            # completion latency, so 8 serialized stores would trail the
            # final matmul by ~8.6us.
            for ot in range(N_OT):
                o_sb = out_pool.tile([P, B_SUPER], F32, tag="outs")
                nc.vector.tensor_scalar(o_sb[:], psums[ot][:], 1.0 / ALPHA,
                                        bias_sb[:, ot:ot + 1],
                                        ALU.mult, ALU.add)
                if bs == N_SUPERS - 1 and ot % 2 == 1:
                    nc.scalar.dma_start(outT[ot, bs], o_sb[:])
                else:
                    nc.gpsimd.dma_start(outT[ot, bs], o_sb[:])

    nc.compile()
    return nc


def _get_nc():
    if "nc" not in _CACHE:
        _CACHE["nc"] = _build()
    return _CACHE["nc"]


def _quantize_ramps(rho, x):
    """Exact 256-path fp8 quantization of ramp weights.

    rho: [O, I, 8] fp64 unscaled ramp weights (sw_j - sw_{j+1}).
    x:   [B, I] fp32 inputs (defines the Gram weighting).

    Returns (q_scaled [O,I,8] e4m3-representable fp32 = approx rho*ALPHA,
             lam [O,I] clip-channel weight, mu [O,I] silu-weight correction,
             dbias [O] bias correction).
    """
    grid = np.linspace(-2.0, 2.0, 9).astype(np.float64)
    # moments under the empirical x distribution (subsampled)
    xr = x.reshape(-1)[::16].astype(np.float64)
    xs = np.clip(xr, -2.0, 2.0)
    H = np.maximum(0.0, 1.0 - np.abs((xs[:, None] - grid) / 0.5))  # [N, 9]
    silu = xr / (1.0 + np.exp(-xr))
    Psi = np.stack([np.ones_like(xr), xs, silu], axis=1)     # [N, 3]
    n = xr.size
    G = (H.T @ H) / n                                        # [9, 9]
    M = (Psi.T @ Psi) / n                                    # [3, 3]
    K = (Psi.T @ H) / n                                      # [3, 9]
    Minv = np.linalg.inv(M)
    Gt = G - K.T @ Minv @ K                                  # residual metric
    # C[j,k] = sum_{m>j, n>k} Gt[m,n]  (P_m = sum_{j<m} err_j)
    Csuf = np.cumsum(np.cumsum(Gt[::-1, ::-1], 0), 1)[::-1, ::-1]
    C = np.ascontiguousarray(Csuf[1:, 1:]).astype(np.float32)

    v = (rho * ALPHA).reshape(-1, 8).astype(np.float32)      # [R, 8]
    q0 = v.astype(E4M3)
    q0f = q0.astype(np.float32)
    up = np.nextafter(q0, np.array(np.inf, dtype=E4M3)).astype(np.float32)
    dn = np.nextafter(q0, np.array(-np.inf, dtype=E4M3)).astype(np.float32)
    lo = np.where(q0f <= v, q0f, dn)
    hi = np.where(q0f >= v, q0f, up)
    e_lo = lo - v
    d = hi - lo                                              # >= 0

    S = ((np.arange(256)[:, None] >> np.arange(8)[None, :]) & 1
         ).astype(np.float32)                                # [256, 8]
    SS = (S[:, :, None] * S[:, None, :]).reshape(256, 64)    # [256, 64]

    R = v.shape[0]
    q = np.empty_like(v)
    CHR = 131072
    for s in range(0, R, CHR):
        el, dd = e_lo[s:s + CHR], d[s:s + CHR]
        u = el @ C                                           # [r, 8]
        lin = 2.0 * dd * u                                   # [r, 8]
        V = (dd[:, :, None] * dd[:, None, :] * C[None]).reshape(-1, 64)
        Q = lin @ S.T + V @ SS.T                             # [r, 256]
        best = Q.argmin(axis=1)
        sel = S[best]                                        # [r, 8]
        q[s:s + CHR] = lo[s:s + CHR] + sel * dd

    q_oi = q.reshape(O, I, 8).astype(np.float64)
    eps = rho - q_oi / ALPHA                                 # unscaled error
    Pm = np.zeros((O, I, 9))
    Pm[..., 1:] = np.cumsum(eps, axis=-1)
    a = np.einsum('rm,oim->oir', Minv @ K, Pm)               # [O, I, 3]
    c, lam, mu = a[..., 0], a[..., 1], a[..., 2]
    T = Pm[..., 8]
    dbias = (0.5 * T - c).sum(axis=1)                        # [O]
    return q.reshape(O, I, 8), lam, mu, dbias


def _stage_inputs(x, spline_weight, base_weight, bias):
    """Host-side input staging shared by kernel() and test harnesses."""
    # x[b, i] -> [core, bs, ichk, p, b_super]
    xt = np.ascontiguousarray(
        x.reshape(N_CORES, N_SUPERS, B_SUPER, N_ICHK, P)
        .transpose(0, 1, 3, 4, 2))

    sw = spline_weight.astype(np.float64)
    rho = sw[..., :8] - sw[..., 1:]                          # [O, I, 8]
    q_scaled, lam, mu, dbias = _quantize_ramps(rho, x)

    # channel 9 of each i-chunk: xc weights = -ALPHA*XCS*lam, e4m3
    xcw = (-ALPHA * XCS * lam).astype(np.float32).astype(E4M3) \
        .astype(np.float32)
    w9 = np.concatenate([q_scaled, xcw[..., None]], axis=2)  # [O, I, 9]
    # wr[p, t2*9+tp, r, o] = w9[o, (2*t2+cc//9)*128+p, cc%9], cc = 2*tp+r
    wr_dev = np.ascontiguousarray(
        w9.reshape(O, N_ICHK, P, 9)
        .transpose(1, 3, 2, 0)                               # [ichk, ch, p, o]
        .reshape(N_T2, 2 * 9, P, O)                          # [t2, cc, p, o]
        .reshape(N_T2, N_PAIR, 2, P, O)
        .transpose(3, 0, 1, 2, 4)                            # [p, t2, tp, r, o]
        .reshape(P, N_T2 * N_PAIR, 2, O).astype(E4M3))

    # wb[p, ichk, o] = 32*(bw - mu)[o, i]
    wb_dev = np.ascontiguousarray(
        (ALPHA * (base_weight.astype(np.float64) - mu))
        .T.reshape(N_ICHK, P, O)
        .transpose(1, 0, 2).astype(NP_BF16))                 # [P, ichk, O]

    # bias fold: bias + 0.5*sum_i(sw0 + sw8) + dbias
    bias_dev = (bias.astype(np.float64)
                + 0.5 * (sw[..., 0].sum(axis=1) + sw[..., 8].sum(axis=1))
                + dbias).astype(np.float32)
    return xt, wr_dev, wb_dev, bias_dev


def kernel(x, spline_weight, base_weight, bias):
    x = np.asarray(x, dtype=np.float32)
    spline_weight = np.asarray(spline_weight, dtype=np.float32)
    base_weight = np.asarray(base_weight, dtype=np.float32)
    bias = np.asarray(bias, dtype=np.float32)

    nc = _get_nc()
    xt, wr_dev, wb_dev, bias_dev = _stage_inputs(
        x, spline_weight, base_weight, bias)

    in_maps = [{"xt": np.ascontiguousarray(xt[c]), "wr": wr_dev,
                "wb": wb_dev, "bias": bias_dev} for c in range(N_CORES)]
    res = run_bass_kernel_spmd(nc, in_maps, core_ids=list(range(N_CORES)))

    # outT[ot, bs, p, b] per core -> out[b, o]
    outs = []
    for c in range(N_CORES):
        oc = np.asarray(res.results[c]["outT"])
        outs.append(oc.transpose(1, 3, 0, 2).reshape(B_CORE, O))
    return np.ascontiguousarray(np.concatenate(outs, axis=0),
                                dtype=np.float32)


# revision 30
# speedup vs baseline: 1.1003x; 1.0698x over previous
"""KAN basis-linear kernel for 8 TRN2 NeuronCores — fp8 DoubleRow edition.

Computes, for x:[B,I], spline_weight:[O,I,K=9], base_weight:[O,I], bias:[O]:

    basis = relu(1 - |(clip(x,-2,2)[...,None] - grid) / delta|)   # hat basis
    out   = einsum('bik,oik->bo', basis, spline_weight)
          + silu(x) @ base_weight.T + bias

Data-parallel over the batch across 8 cores (weights replicated).

Algebra: with grid g_k = -2 + 0.5k, Abel summation turns the hat-basis
contraction into 8 saturating-ramp channels psi_j = clip(2(g_{j+1}-x),0,1);
shifting to phi'_j = psi_j - 0.5 (saturated values +-0.5, exactly
representable in fp8) folds a constant into the bias. The 8 ramp channels
run as fp8(e4m3) DoubleRow matmuls (2 contraction rows/cycle = 2x bf16
rate), pairing the 16 channels of each two-i-chunk phi tile into 8 pairs;
the silu base branch stays bf16.

fp8 weight quantization is chosen per (o,i) by exact 256-path search: the
output error equals sum_i [P_i(x) - absorber_i(x)] where P_i linearly
interpolates the cumulative quantization errors over the 8 knots; each
(o,i) row has 2^8 floor/ceil choices, scored by a Gram-metric quadratic
form under the empirical x distribution. The best absorber component over
{1, silu(x)} is folded into the bias and the silu channel weights (mu).
"""
import numpy as np
import ml_dtypes
from contextlib import ExitStack

import concourse.bass as bass
import concourse.tile as tile
import concourse.mybir as mybir
from concourse import bacc
from concourse.bass_utils import run_bass_kernel_spmd

N_CORES = 8
B, I, O = 16384, 1024, 1024
B_CORE = B // N_CORES            # 2048 batch rows per core
B_SUPER = 512                    # batch stripe held in PSUM (1 bank per o-tile)
N_SUPERS = B_CORE // B_SUPER     # 4
P = 128
N_ICHK = 8                       # contraction chunks over i
N_T2 = 4                         # phi tiles per stripe (2 i-chunks each)
N_PAIR = 8                       # DoubleRow pairs per 2-ichk tile (16 channels)
N_OT = O // P                    # 8 output tiles (one PSUM bank each)
ALPHA = 32.0                     # fp8 weight scale (psum divided back on evac)

F32 = mybir.dt.float32
BF16 = mybir.dt.bfloat16
F8 = mybir.dt.float8e4
E4M3 = ml_dtypes.float8_e4m3
NP_BF16 = ml_dtypes.bfloat16
AF = mybir.ActivationFunctionType
ALU = mybir.AluOpType
DR = mybir.MatmulPerfMode.DoubleRow

_CACHE = {}


def _build():
    nc = bacc.Bacc("TRN2", target_bir_lowering=False, debug=False,
                   num_devices=N_CORES)
    # x tiled on host: [bs, ichk, p, b]
    xt = nc.dram_tensor("xt", [N_SUPERS, N_ICHK, P, B_SUPER], F32,
                        kind="ExternalInput").ap()
    # fp8 weights: [p, 32 pairs, 2, o]; pair t2*8+tp slot r is ramp channel
    # cc=2*tp+r of tile t2: (ichk=2*t2+cc//8, j=cc%8)
    wr = nc.dram_tensor("wr", [P, N_T2 * N_PAIR, 2, O], F8,
                        kind="ExternalInput").ap()
    # bf16 silu weights: [p, ichk, o]
    wb = nc.dram_tensor("wb", [P, N_ICHK, O], BF16,
                        kind="ExternalInput").ap()
    bias = nc.dram_tensor("bias", [O], F32, kind="ExternalInput").ap()
    # output tiled: [ot, bs, p, b] (contiguous 256KB stores)
    outT = nc.dram_tensor("outT", [N_OT, N_SUPERS, P, B_SUPER], F32,
                          kind="ExternalOutput").ap()

    with tile.TileContext(nc) as tc, ExitStack() as ctx:
        const_pool = ctx.enter_context(tc.tile_pool(name="const", bufs=1))
        x_pool = ctx.enter_context(tc.tile_pool(name="xin", bufs=3))
        t_pool = ctx.enter_context(tc.tile_pool(name="tmp", bufs=4))
        phi8_pool = ctx.enter_context(tc.tile_pool(name="phi8", bufs=3))
        phib_pool = ctx.enter_context(tc.tile_pool(name="phib", bufs=4))
        w_pool = ctx.enter_context(tc.tile_pool(name="wts", bufs=1))
        out_pool = ctx.enter_context(tc.tile_pool(name="outs", bufs=3))
        psum_pool = ctx.enter_context(
            tc.tile_pool(name="psum", bufs=N_OT, space="PSUM"))

        # first x chunk ASAP (phi production is the kernel's lead-in)
        x_first = x_pool.tile([P, B_SUPER], F32, tag="xin")
        nc.scalar.dma_start(x_first[:], xt[0, 0])

        # ACT bias constants: 2*g_{j+1} = j - 3 for j=0..7
        consts = const_pool.tile([P, 8], F32)
        for j in range(8):
            nc.any.memset(consts[:, j:j + 1], float(j - 3))

        # bias[o] -> [128, 8] with o = ot*128 + p
        bias_sb = const_pool.tile([P, N_OT], F32)
        nc.scalar.dma_start(bias_sb[:], bias.rearrange("(ot p) -> p ot", p=P))

        # resident weights, streamed in consumption order on the sync queue
        wr_sb = w_pool.tile([P, N_T2 * N_PAIR, 2, O], F8)
        wb_sb = w_pool.tile([P, N_ICHK, O], BF16)
        for tp in range(N_PAIR):  # first tile per-pair so matmul 0 starts early
            nc.sync.dma_start(wr_sb[:, tp:tp + 1], wr[:, tp:tp + 1])
        nc.sync.dma_start(wb_sb[:, 0:2], wb[:, 0:2])
        for t2 in range(1, N_T2):
            nc.sync.dma_start(wr_sb[:, t2 * N_PAIR:(t2 + 1) * N_PAIR],
                              wr[:, t2 * N_PAIR:(t2 + 1) * N_PAIR])
            nc.sync.dma_start(wb_sb[:, 2 * t2:2 * t2 + 2], wb[:, 2 * t2:2 * t2 + 2])


        # Small PE warm-up spin bridging the first input-DMA wait: starts
        # the HAM busy-streak early so the clock-gate reaches 8/8 sooner.
        warm = const_pool.tile([P, B_SUPER], BF16)
        nc.any.memset(warm[:], 0.0)
        warm_ps = psum_pool.tile([P, B_SUPER], F32, tag="psum")
        for _ in range(11):
            nc.tensor.matmul(warm_ps[:], lhsT=warm[:, :P], rhs=warm[:],
                             start=True, stop=True)

        for bs in range(N_SUPERS):
            psums = [psum_pool.tile([P, B_SUPER], F32, tag="psum",
                                    name=f"psum_{bs}_{ot}")
                     for ot in range(N_OT)]
            phibs = {}
            for t2 in range(N_T2):
                # ---- phi production for the 2 i-chunks of this tile ----
                phi8 = phi8_pool.tile([P, 2 * N_PAIR, B_SUPER], F8, tag="phi8")
                for sub in range(2):
                    ichk = 2 * t2 + sub
                    if bs == 0 and ichk == 0:
                        x_sb = x_first
                    else:
                        x_sb = x_pool.tile([P, B_SUPER], F32, tag="xin")
                        nc.scalar.dma_start(x_sb[:], xt[bs, ichk])
                    base = sub * N_PAIR
                    for j in range(8):
                        t = t_pool.tile([P, B_SUPER], F32, tag="tmp")
                        nc.scalar.activation(t[:], x_sb[:], AF.Relu,
                                             bias=consts[:, j:j + 1],
                                             scale=-2.0)
                        nc.vector.tensor_scalar(phi8[:, base + j, :], t[:],
                                                1.0, -0.5, ALU.min, ALU.add)
                    # silu channel (bf16)
                    phib = phib_pool.tile([P, B_SUPER], BF16, tag="phib",
                                          name=f"phib_{bs}_{ichk}")
                    nc.scalar.activation(phib[:], x_sb[:], AF.Silu)
                    phibs[ichk] = phib

                # ---- matmuls ----
                last_tile = (t2 == N_T2 - 1)
                if last_tile:
                    # finish banks one at a time so evacuation and output
                    # DMA overlap the remaining matmuls
                    for ot in range(N_OT):
                        for tp in range(N_PAIR):
                            nc.tensor.matmul(
                                psums[ot][:],
                                lhsT=wr_sb[:, t2 * N_PAIR + tp, :,
                                           bass.ts(ot, P)],
                                rhs=phi8[:, 2 * tp:2 * tp + 2, :],
                                perf_mode=DR, start=False, stop=False)
                        for sub in range(2):
                            nc.tensor.matmul(
                                psums[ot][:],
                                lhsT=wb_sb[:, 2 * t2 + sub, bass.ts(ot, P)],
                                rhs=phibs[2 * t2 + sub][:],
                                start=False, stop=(sub == 1))
                else:
                    for tp in range(N_PAIR):
                        for ot in range(N_OT):
                            nc.tensor.matmul(
                                psums[ot][:],
                                lhsT=wr_sb[:, t2 * N_PAIR + tp, :,
                                           bass.ts(ot, P)],
                                rhs=phi8[:, 2 * tp:2 * tp + 2, :],
                                perf_mode=DR,
                                start=(t2 == 0 and tp == 0), stop=False)
                    for sub in range(2):
                        for ot in range(N_OT):
                            nc.tensor.matmul(
                                psums[ot][:],
                                lhsT=wb_sb[:, 2 * t2 + sub, bass.ts(ot, P)],
                                rhs=phibs[2 * t2 + sub][:],
                                start=False, stop=False)

            # evacuate PSUM: out = psum/ALPHA + bias (DVE). The last stripe's
            # stores alternate across two queues: each dma_start carries ~1us
            # completion latency, so 8 serialized stores would trail the
            # final matmul by ~8.6us.
            for ot in range(N_OT):
                o_sb = out_pool.tile([P, B_SUPER], F32, tag="outs")
                nc.vector.tensor_scalar(o_sb[:], psums[ot][:], 1.0 / ALPHA,
                                        bias_sb[:, ot:ot + 1],
                                        ALU.mult, ALU.add)
                if bs == N_SUPERS - 1 and ot % 2 == 1:
                    nc.scalar.dma_start(outT[ot, bs], o_sb[:])
                else:
                    nc.gpsimd.dma_start(outT[ot, bs], o_sb[:])

    nc.compile()
    return nc


def _get_nc():
    if "nc" not in _CACHE:
        _CACHE["nc"] = _build()
    return _CACHE["nc"]


def _quantize_ramps(rho, x):
    """Exact 256-path fp8 quantization of ramp weights.

    rho: [O, I, 8] fp64 unscaled ramp weights (sw_j - sw_{j+1}).
    x:   [B, I] fp32 inputs (defines the Gram weighting).

    Returns (q_scaled [O,I,8] e4m3-representable fp32 = approx rho*ALPHA,
             lam [O,I] clip-channel weight, mu [O,I] silu-weight correction,
             dbias [O] bias correction).
    """
    grid = np.linspace(-2.0, 2.0, 9).astype(np.float64)
    # moments under the empirical x distribution (subsampled)
    xr = x.reshape(-1)[::16].astype(np.float64)
    xs = np.clip(xr, -2.0, 2.0)
    H = np.maximum(0.0, 1.0 - np.abs((xs[:, None] - grid) / 0.5))  # [N, 9]
    silu = xr / (1.0 + np.exp(-xr))
    Psi = np.stack([np.ones_like(xr), silu], axis=1)         # [N, 2]
    n = xr.size
    G = (H.T @ H) / n                                        # [9, 9]
    M = (Psi.T @ Psi) / n                                    # [3, 3]
    K = (Psi.T @ H) / n                                      # [3, 9]
    Minv = np.linalg.inv(M)
    Gt = G - K.T @ Minv @ K                                  # residual metric
    # C[j,k] = sum_{m>j, n>k} Gt[m,n]  (P_m = sum_{j<m} err_j)
    Csuf = np.cumsum(np.cumsum(Gt[::-1, ::-1], 0), 1)[::-1, ::-1]
    C = np.ascontiguousarray(Csuf[1:, 1:]).astype(np.float32)

    v = (rho * ALPHA).reshape(-1, 8).astype(np.float32)      # [R, 8]
    q0 = v.astype(E4M3)
    q0f = q0.astype(np.float32)
    up = np.nextafter(q0, np.array(np.inf, dtype=E4M3)).astype(np.float32)
    dn = np.nextafter(q0, np.array(-np.inf, dtype=E4M3)).astype(np.float32)
    lo = np.where(q0f <= v, q0f, dn)
    hi = np.where(q0f >= v, q0f, up)
    e_lo = lo - v
    d = hi - lo                                              # >= 0

    S = ((np.arange(256)[:, None] >> np.arange(8)[None, :]) & 1
         ).astype(np.float32)                                # [256, 8]
    SS = (S[:, :, None] * S[:, None, :]).reshape(256, 64)    # [256, 64]

    R = v.shape[0]
    q = np.empty_like(v)
    CHR = 131072
    for s in range(0, R, CHR):
        el, dd = e_lo[s:s + CHR], d[s:s + CHR]
        u = el @ C                                           # [r, 8]
        lin = 2.0 * dd * u                                   # [r, 8]
        V = (dd[:, :, None] * dd[:, None, :] * C[None]).reshape(-1, 64)
        Q = lin @ S.T + V @ SS.T                             # [r, 256]
        best = Q.argmin(axis=1)
        sel = S[best]                                        # [r, 8]
        q[s:s + CHR] = lo[s:s + CHR] + sel * dd

    q_oi = q.reshape(O, I, 8).astype(np.float64)
    eps = rho - q_oi / ALPHA                                 # unscaled error
    Pm = np.zeros((O, I, 9))
    Pm[..., 1:] = np.cumsum(eps, axis=-1)
    a = np.einsum('rm,oim->oir', Minv @ K, Pm)               # [O, I, 2]
    c, mu = a[..., 0], a[..., 1]
    T = Pm[..., 8]
    dbias = (0.5 * T - c).sum(axis=1)                        # [O]
    return q.reshape(O, I, 8), mu, dbias


def _stage_inputs(x, spline_weight, base_weight, bias):
    """Host-side input staging shared by kernel() and test harnesses."""
    # x[b, i] -> [core, bs, ichk, p, b_super]
    xt = np.ascontiguousarray(
        x.reshape(N_CORES, N_SUPERS, B_SUPER, N_ICHK, P)
        .transpose(0, 1, 3, 4, 2))

    sw = spline_weight.astype(np.float64)
    rho = sw[..., :8] - sw[..., 1:]                          # [O, I, 8]
    q_scaled, mu, dbias = _quantize_ramps(rho, x)

    # wr[p, t2*8+tp, r, o] = q[o, (2*t2+cc//8)*128+p, cc%8], cc = 2*tp+r
    wr_dev = np.ascontiguousarray(
        q_scaled.reshape(O, N_ICHK, P, 8)
        .transpose(1, 3, 2, 0)                               # [ichk, j, p, o]
        .reshape(N_T2, 2 * 8, P, O)                          # [t2, cc, p, o]
        .reshape(N_T2, N_PAIR, 2, P, O)
        .transpose(3, 0, 1, 2, 4)                            # [p, t2, tp, r, o]
        .reshape(P, N_T2 * N_PAIR, 2, O).astype(E4M3))

    # wb[p, ichk, o] = 32*(bw - mu)[o, i]
    wb_dev = np.ascontiguousarray(
        (ALPHA * (base_weight.astype(np.float64) - mu))
        .T.reshape(N_ICHK, P, O)
        .transpose(1, 0, 2).astype(NP_BF16))                 # [P, ichk, O]

    # bias fold: bias + 0.5*sum_i(sw0 + sw8) + dbias
    bias_dev = (bias.astype(np.float64)
                + 0.5 * (sw[..., 0].sum(axis=1) + sw[..., 8].sum(axis=1))
                + dbias).astype(np.float32)
    return xt, wr_dev, wb_dev, bias_dev


def kernel(x, spline_weight, base_weight, bias):
    x = np.asarray(x, dtype=np.float32)
    spline_weight = np.asarray(spline_weight, dtype=np.float32)
    base_weight = np.asarray(base_weight, dtype=np.float32)
    bias = np.asarray(bias, dtype=np.float32)

    nc = _get_nc()
    xt, wr_dev, wb_dev, bias_dev = _stage_inputs(
        x, spline_weight, base_weight, bias)

    in_maps = [{"xt": np.ascontiguousarray(xt[c]), "wr": wr_dev,
                "wb": wb_dev, "bias": bias_dev} for c in range(N_CORES)]
    res = run_bass_kernel_spmd(nc, in_maps, core_ids=list(range(N_CORES)))

    # outT[ot, bs, p, b] per core -> out[b, o]
    outs = []
    for c in range(N_CORES):
        oc = np.asarray(res.results[c]["outT"])
        outs.append(oc.transpose(1, 3, 0, 2).reshape(B_CORE, O))
    return np.ascontiguousarray(np.concatenate(outs, axis=0),
                                dtype=np.float32)
